# revision 7
# baseline (speedup 1.0000x reference)
"""Causal self-attention on 8 Trainium2 NeuronCores.

Sharding: 8 cores = 4 batches x 2 head-groups (8 heads each).
Each core runs an identical SPMD program:
  - QKV projections for its head group (weights pre-transposed + bf16 on host)
  - causal attention computed in transposed-score layout S^T[s, t] so the
    AV matmul consumes P^T directly (no on-chip transposes at all)
  - softmax denominators come for free from a ones-column appended to V
  - row-sharded Wo projection produces a partial output; the two cores of a
    batch are summed on the host during unsharding.

Schedule: the two heads of a pair run their QK^T matmuls as K=64 matmuls
packed into disjoint PE row-halves (tile_position (0,0) / (64,0)); the HW
runs same-row-group-disjoint matmuls concurrently, so a packed pair costs
one matmul of streaming time.  AV trails the exp by AVLAG steps and
projection/output-projection "filler" matmuls keep the PE stream dense
while ScalarE computes the softmax exps.  The softmax denominator
reciprocals are broadcast across partitions with a tiny K=2 selector
matmul (no DRAM bounce), so the last chunk's normalize no longer blocks
the final output projection on DMA round trips.

B=4, T=2048, D=1024, H=16, dh=64.
"""

import numpy as np
import ml_dtypes

B, T, D = 4, 2048, 1024
P = 128
KD = D // P  # 8 contraction tiles for the input dim
HL = 8  # heads per core
HP = HL // 2  # head pairs per core (pair shares a 128-partition tile)
DH = 64
TCH = 512  # t-chunk (psum bank width in fp32)
NC4 = T // TCH  # 4 chunks
NTT = T // P  # 16 t-tiles
AVLAG = 2  # AV trails QK^T by this many s-tiles (hides exp latency)

_CACHE = {}


def _split_waits(nc, mybir, limit=1):
    """walrus in this container accepts at most one sem-wait per instruction;
    hoist extra waits onto preceding NoOps on the same engine."""
    cnt = 0
    for bb in nc.main_func.blocks:
        newlist = []
        for inst in bb.instructions:
            si = inst.sync_info
            if si is not None and len(si.on_wait) > limit:
                waits = list(si.on_wait)
                extra, keep = waits[:-limit], waits[-limit:]
                for w in extra:
                    cnt += 1
                    nop = mybir.InstNoOp(name=f"WSPLIT-{cnt}")
                    nop.engine = inst.engine
                    nop.sync_info = mybir.SyncInfo(on_wait=[w], on_update=[])
                    newlist.append(nop)
                inst.sync_info = mybir.SyncInfo(
                    on_wait=keep, on_update=list(si.on_update)
                )
            newlist.append(inst)
        bb.instructions[:] = newlist
    return cnt


def _build():
    if "nc" in _CACHE:
        return _CACHE["nc"]

    from contextlib import ExitStack

    import concourse.bass as bass
    import concourse.tile as tile
    from concourse import mybir

    f32 = mybir.dt.float32
    bf = mybir.dt.bfloat16
    Exp = mybir.ActivationFunctionType.Exp

    nc = bass.Bass()
    xT = nc.declare_dram_parameter("xT", [D, T], bf, isOutput=False)
    wq = nc.declare_dram_parameter("wq", [D, HL * DH], bf, isOutput=False)
    wk = nc.declare_dram_parameter("wk", [D, HL * DH], bf, isOutput=False)
    wv = nc.declare_dram_parameter("wv", [D, HL * DH], bf, isOutput=False)
    wo = nc.declare_dram_parameter("wo", [HL * DH, D], bf, isOutput=False)
    mk = nc.declare_dram_parameter("mask", [P, P], bf, isOutput=False)
    out = nc.declare_dram_parameter("out", [T, D], f32, isOutput=True)

    with tile.TileContext(nc) as tc, ExitStack() as ctx:
        psum = ctx.enter_context(tc.tile_pool(name="psum", bufs=1, space="PSUM"))
        per = ctx.enter_context(tc.tile_pool(name="per", bufs=1))

        # per-k weight tiles + chunk-split x so the first projection only
        # waits on ~2 MB of DMA instead of the whole 8.4 MB load
        wq_sb = [per.tile([P, HL * DH], bf, name=f"wq_{k}") for k in range(KD)]
        wk_sb = [per.tile([P, HL * DH], bf, name=f"wk_{k}") for k in range(KD)]
        wv_sb = [per.tile([P, HL * DH], bf, name=f"wv_{k}") for k in range(KD)]
        wo_sb = [per.tile([P, D], bf, name=f"wo_{k}") for k in range(HL * DH // P)]
        mk_sb = per.tile([P, P], bf)
        # ones row for the K=1 denominator-broadcast matmuls
        ones_sb = per.tile([1, 64], bf, name="ones")
        xc0_sb = [per.tile([P, TCH], bf, name=f"xc0_{k}") for k in range(KD)]
        xr_sb = [per.tile([P, 3 * TCH], bf, name=f"xr_{k}") for k in range(KD)]

        # Q^T per head pair: partitions 0:64 = even head dims, 64:128 = odd.
        # All per-index tiles so Tile's dependency tracking stays exact.
        qt_sb = {
            (m, cc): per.tile([P, TCH], bf, name=f"qt_{m}_{cc}")
            for m in range(HP)
            for cc in range(NC4)
        }
        kt_sb = {
            (m, cc): per.tile([P, TCH], bf, name=f"kt_{m}_{cc}")
            for m in range(HP)
            for cc in range(NC4)
        }
        v_sb = [per.tile([P, HL, 66], bf, name=f"v_{tt}") for tt in range(NTT)]
        yt_sb = {
            (m, cc): per.tile([P, TCH], bf, name=f"yt_{m}_{cc}")
            for m in range(HP)
            for cc in range(NC4)
        }

        # ---- loads, ordered so chunk-0 QKV proj can start early ----
        nc.sync.dma_start(out=mk_sb[:], in_=mk[:, :])
        for k in range(KD):
            nc.sync.dma_start(
                out=xc0_sb[k][:], in_=xT[k * P : (k + 1) * P, 0:TCH]
            )
            nc.sync.dma_start(out=wq_sb[k][:], in_=wq[k * P : (k + 1) * P, :])
        for k in range(KD):
            nc.sync.dma_start(out=wk_sb[k][:], in_=wk[k * P : (k + 1) * P, :])
        for k in range(KD):
            nc.sync.dma_start(out=wv_sb[k][:], in_=wv[k * P : (k + 1) * P, :])
        for k in range(KD):
            nc.sync.dma_start(
                out=xr_sb[k][:], in_=xT[k * P : (k + 1) * P, TCH:T]
            )
        for k in range(HL * DH // P):
            nc.sync.dma_start(out=wo_sb[k][:], in_=wo[k * P : (k + 1) * P, :])

        nc.vector.memset(ones_sb[:, :], 1.0)
        for tt in range(NTT):
            nc.vector.memset(v_sb[tt][:, :, 64:65], 1.0)

        def xap(k, cc, c0, c1):
            """x^T tile slice for contraction tile k, columns [c0,c1) of
            chunk cc."""
            if cc == 0:
                return xc0_sb[k][:, c0:c1]
            base = (cc - 1) * TCH
            return xr_sb[k][:, base + c0 : base + c1]

        pt_pool = ctx.enter_context(tc.tile_pool(name="ptp", bufs=8))
        sm_pool = ctx.enter_context(tc.tile_pool(name="smp", bufs=4))
        out_pool = ctx.enter_context(tc.tile_pool(name="outp", bufs=2))

        def gen_proj(cc):
            """QKV projections for chunk cc; yields after each matmul."""
            for m in range(HP):
                msl = slice(m * P, (m + 1) * P)
                pq = psum.tile([P, TCH], f32, tag="pp", bufs=2, name=f"pq{cc}_{m}")
                for k in range(KD):
                    nc.tensor.matmul(
                        out=pq[:],
                        lhsT=wq_sb[k][:, msl],
                        rhs=xap(k, cc, 0, TCH),
                        start=(k == 0),
                        stop=(k == KD - 1),
                    )
                    yield
                nc.vector.tensor_copy(out=qt_sb[(m, cc)][:, :], in_=pq[:])
                pk = psum.tile([P, TCH], f32, tag="pp", bufs=2, name=f"pk{cc}_{m}")
                for k in range(KD):
                    nc.tensor.matmul(
                        out=pk[:],
                        lhsT=wk_sb[k][:, msl],
                        rhs=xap(k, cc, 0, TCH),
                        start=(k == 0),
                        stop=(k == KD - 1),
                    )
                    yield
                nc.vector.tensor_copy(out=kt_sb[(m, cc)][:, :], in_=pk[:])
            for tt in range(4 * cc, 4 * cc + 4):
                tl = tt - 4 * cc
                pv = psum.tile([P, TCH], f32, tag="pp", bufs=2, name=f"pv{tt}")
                for k in range(KD):
                    nc.tensor.matmul(
                        out=pv[:],
                        lhsT=xap(k, cc, tl * P, (tl + 1) * P),
                        rhs=wv_sb[k][:],
                        start=(k == 0),
                        stop=(k == KD - 1),
                    )
                    yield
                nc.vector.tensor_copy(
                    out=v_sb[tt][:, :, 0:64],
                    in_=pv.rearrange("p (h d) -> p h d", h=HL),
                )

        def gen_oproj(chunks, korder=None, dma_eng=None):
            """Output projection for the given chunks; yields per matmul."""
            ks = list(korder or range(HL * DH // P))
            for c2 in chunks:
                for tt in range(4 * c2, 4 * c2 + 4):
                    ob = out_pool.tile([P, D], f32, tag="ob", name=f"ob{tt}")
                    for n2 in range(2):
                        po = psum.tile(
                            [P, TCH], f32, tag="pp", bufs=2, name=f"po{tt}_{n2}"
                        )
                        for i, k in enumerate(ks):
                            tl = tt - 4 * c2
                            nc.tensor.matmul(
                                out=po[:],
                                lhsT=yt_sb[(k, c2)][:, tl * P : (tl + 1) * P],
                                rhs=wo_sb[k][:, n2 * TCH : (n2 + 1) * TCH],
                                start=(i == 0),
                                stop=(i == len(ks) - 1),
                            )
                            yield
                        nc.vector.tensor_copy(
                            out=ob[:, n2 * TCH : (n2 + 1) * TCH], in_=po[:]
                        )
                    (dma_eng or nc.sync).dma_start(
                        out=out[tt * P : (tt + 1) * P, :], in_=ob[:]
                    )

        # projections for chunk 0 run unzipped up front (also warms the PE)
        for _ in gen_proj(0):
            pass

        # Global filler queue: a list of (tag, generator) consumed ~2 matmuls
        # per attention step; before attention chunk c its projections must be
        # fully emitted (Tile orders by program order), so drain through the
        # matching tag at each chunk start. O-proj generators are appended as
        # soon as their chunk's attention completes.
        fillq = [(("proj", cc), gen_proj(cc)) for cc in range(1, NC4)]

        def fill(n):
            done = 0
            while done < n and fillq:
                try:
                    next(fillq[0][1])
                    done += 1
                except StopIteration:
                    fillq.pop(0)
            return done

        def drain_through(tag):
            while fillq and any(t == tag for t, _ in fillq):
                try:
                    next(fillq[0][1])
                except StopIteration:
                    fillq.pop(0)

        FILL_PER_STEP = 2

        # ---- attention: per chunk, all head pairs, with filler zipped in ----
        for c in range(NC4):
            n_st = 4 * c + 4
            drain_through(("proj", c))

            # last chunk: run hp=3 first so the final O-proj (k emitted in the
            # same rotated order) is never blocked on the last norm chain
            hporder = [3, 0, 1, 2] if c == NC4 - 1 else list(range(HP))
            for hp in hporder:
                pts = {}
                psys = {}

                def emit_av(st, hp=hp, pts=pts, psys=psys, n_st=n_st):
                    pt, base, lo = pts[st]
                    for par in (0, 1):
                        if st == 0:
                            psys[par] = psum.tile(
                                [65, TCH], f32, tag="py", bufs=2, name=f"psy{par}"
                            )
                        nc.tensor.matmul(
                            out=psys[par][:, lo:TCH],
                            lhsT=v_sb[st][:, 2 * hp + par, 0:65],
                            rhs=pt[:, base + par, lo:TCH],
                            start=(st == 0),
                            stop=(st == n_st - 1),
                        )

                for st in range(n_st):
                    kd = st - 4 * c  # >=0 on causal-diagonal s-tiles
                    lo = max(kd, 0) * P
                    pss = psum.tile([P, 2 * TCH], f32, tag="ps2", bufs=2, name="pss")
                    # both heads of the pair packed into disjoint PE
                    # row-halves -> the two matmuls run concurrently
                    for par in (0, 1):
                        rows = slice(64 * par, 64 * par + 64)
                        nc.tensor.matmul(
                            out=pss[:, par * TCH + lo : (par + 1) * TCH],
                            lhsT=kt_sb[(hp, st // 4)][
                                rows, (st % 4) * P : (st % 4 + 1) * P
                            ],
                            rhs=qt_sb[(hp, c)][rows, lo:TCH],
                            start=True,
                            stop=True,
                            tile_position=(64 * par, 0),
                        )
                    pt = pt_pool.tile([P, 2, TCH], bf, tag="pt", name="pt")
                    nc.scalar.activation(
                        out=pt[:, :, lo:TCH],
                        in_=pss.rearrange("p (a b) -> p a b", a=2)[:, :, lo:TCH],
                        func=Exp,
                        scale=1.0 / np.sqrt(DH),
                    )
                    if kd >= 0:
                        for par in (0, 1):
                            nc.vector.tensor_mul(
                                pt[:, par, lo : lo + P],
                                pt[:, par, lo : lo + P],
                                mk_sb[:],
                            )
                    pts[st] = (pt, 0, lo)
                    if st >= AVLAG:
                        emit_av(st - AVLAG)
                    fill(FILL_PER_STEP)
                for st in range(n_st - AVLAG, n_st):
                    emit_av(st)

                # normalize: y^T = psy[0:64] * (1/psy[64]) with the
                # reciprocal row broadcast across partitions by K=1 matmuls
                # col-tiled to (0,0)/(0,64) -- everything stays on-chip.
                bc = psum.tile([P, TCH], f32, tag="pp", bufs=2, name="bc")
                with nc.allow_low_precision(reason="bf16 recip feeds bcast mm"):
                    for par in (0, 1):
                        rr = sm_pool.tile([1, TCH], bf, tag=f"rr{par}", name="rr")
                        nc.vector.reciprocal(
                            out=rr[:, :], in_=psys[par][64:65, :]
                        )
                        nc.tensor.matmul(
                            out=bc[64 * par : 64 * par + 64, :],
                            lhsT=ones_sb[:, :],
                            rhs=rr[:, :],
                            start=True,
                            stop=True,
                            tile_position=(0, 64 * par),
                        )
                rb = sm_pool.tile([P, TCH], f32, tag="rb", name="rb")
                nc.vector.tensor_copy(out=rb[:, :], in_=bc[:, :])
                for par in (0, 1):
                    rows = slice(64 * par, 64 * par + 64)
                    nc.vector.tensor_mul(
                        yt_sb[(hp, c)][rows, :],
                        psys[par][0:64, :],
                        rb[rows, :],
                    )
            # this chunk's output projection becomes available filler
            if c < NC4 - 1:
                fillq.append((("oproj", c), gen_oproj([c])))

        # drain remaining filler, then the last chunk's output projection
        while fill(64):
            pass
        for _ in gen_oproj([3], korder=[3, 0, 1, 2], dma_eng=nc.scalar):
            pass

    _split_waits(nc, mybir, 1)
    _CACHE["nc"] = nc
    return nc


def kernel(x, Wq, Wk, Wv, Wo):
    from concourse.bass_utils import run_bass_kernel_spmd

    nc = _build()
    bf16 = ml_dtypes.bfloat16

    band = np.tril(np.ones((P, P), np.float32)).T.astype(bf16)  # band[s,j]=s<=j
    xTs = [np.ascontiguousarray(x[b].T).astype(bf16) for b in range(B)]
    in_maps = []
    for c in range(8):
        b, hg = divmod(c, 2)
        sl = slice(512 * hg, 512 * hg + 512)
        in_maps.append(
            {
                "xT": xTs[b],
                "wq": np.ascontiguousarray(Wq[sl, :].T).astype(bf16),
                "wk": np.ascontiguousarray(Wk[sl, :].T).astype(bf16),
                "wv": np.ascontiguousarray(Wv[sl, :].T).astype(bf16),
                "wo": np.ascontiguousarray(Wo[:, sl].T).astype(bf16),
                "mask": band,
            }
        )
    res = None
    for attempt in range(4):
        try:
            res = run_bass_kernel_spmd(nc, in_maps, list(range(8)))
            break
        except Exception:
            # transient NRT_EXEC_UNIT_UNRECOVERABLE has been observed on the
            # first execution of a freshly loaded NEFF; retry a few times
            if attempt == 3:
                raise
            import time

            time.sleep(3)
    _CACHE["exec_time_ns"] = res.exec_time_ns
    outp = np.empty((B, T, D), np.float32)
    for b in range(B):
        outp[b] = res.results[2 * b]["out"] + res.results[2 * b + 1]["out"]
    return outp


# revision 10
# speedup vs baseline: 1.2668x; 1.2668x over previous
"""Causal self-attention on 8 Trainium2 NeuronCores.

Sharding: 8 cores = 4 batches x 2 head-groups (8 heads each).
Each core runs an identical SPMD program:
  - QKV projections for its head group (weights pre-transposed + bf16 on host)
  - causal attention computed in transposed-score layout S^T[s, t] so the
    AV matmul consumes P^T directly (no on-chip transposes at all)
  - softmax denominators come for free from a ones-column appended to V
  - row-sharded Wo projection produces a partial output; the two cores of a
    batch are summed on the host during unsharding.

Schedule: the two heads of a pair run their QK^T matmuls as K=64 matmuls
packed into disjoint PE row-halves (tile_position (0,0) / (64,0)); the HW
runs same-row-group-disjoint matmuls concurrently, so a packed pair costs
one matmul of streaming time.  AV trails the exp by AVLAG steps and
projection/output-projection "filler" matmuls keep the PE stream dense
while ScalarE computes the softmax exps.  The softmax denominator
reciprocals are broadcast across partitions with a tiny K=2 selector
matmul (no DRAM bounce), so the last chunk's normalize no longer blocks
the final output projection on DMA round trips.

B=4, T=2048, D=1024, H=16, dh=64.
"""

import numpy as np
import ml_dtypes

B, T, D = 4, 2048, 1024
P = 128
KD = D // P  # 8 contraction tiles for the input dim
HL = 8  # heads per core
HP = HL // 2  # head pairs per core (pair shares a 128-partition tile)
DH = 64
TCH = 512  # t-chunk (psum bank width in fp32)
NC4 = T // TCH  # 4 chunks
NTT = T // P  # 16 t-tiles
AVLAG = 2  # AV trails QK^T by this many s-tiles (hides exp latency)

_CACHE = {}


def _split_waits(nc, mybir, limit=1):
    """walrus in this container accepts at most one sem-wait per instruction;
    hoist extra waits onto preceding NoOps on the same engine."""
    cnt = 0
    for bb in nc.main_func.blocks:
        newlist = []
        for inst in bb.instructions:
            si = inst.sync_info
            if si is not None and len(si.on_wait) > limit:
                waits = list(si.on_wait)
                extra, keep = waits[:-limit], waits[-limit:]
                for w in extra:
                    cnt += 1
                    nop = mybir.InstNoOp(name=f"WSPLIT-{cnt}")
                    nop.engine = inst.engine
                    nop.sync_info = mybir.SyncInfo(on_wait=[w], on_update=[])
                    newlist.append(nop)
                inst.sync_info = mybir.SyncInfo(
                    on_wait=keep, on_update=list(si.on_update)
                )
            newlist.append(inst)
        bb.instructions[:] = newlist
    return cnt


def _build():
    if "nc" in _CACHE:
        return _CACHE["nc"]

    from contextlib import ExitStack

    import concourse.bass as bass
    import concourse.tile as tile
    from concourse import mybir

    f32 = mybir.dt.float32
    bf = mybir.dt.bfloat16
    Exp = mybir.ActivationFunctionType.Exp

    nc = bass.Bass()
    xT = nc.declare_dram_parameter("xT", [D, T], bf, isOutput=False)
    wq = nc.declare_dram_parameter("wq", [D, HL * DH], bf, isOutput=False)
    wk = nc.declare_dram_parameter("wk", [D, HL * DH], bf, isOutput=False)
    wv = nc.declare_dram_parameter("wv", [D, HL * DH], bf, isOutput=False)
    wo = nc.declare_dram_parameter("wo", [HL * DH, D], bf, isOutput=False)
    mk = nc.declare_dram_parameter("mask", [P, P], bf, isOutput=False)
    out = nc.declare_dram_parameter("out", [T, D], f32, isOutput=True)

    with tile.TileContext(nc) as tc, ExitStack() as ctx:
        psum = ctx.enter_context(tc.tile_pool(name="psum", bufs=1, space="PSUM"))
        per = ctx.enter_context(tc.tile_pool(name="per", bufs=1))

        # per-k weight tiles + chunk-split x so the first projection only
        # waits on ~2 MB of DMA instead of the whole 8.4 MB load
        wq_sb = [per.tile([P, HL * DH], bf, name=f"wq_{k}") for k in range(KD)]
        wk_sb = [per.tile([P, HL * DH], bf, name=f"wk_{k}") for k in range(KD)]
        wv_sb = [per.tile([P, HL * DH], bf, name=f"wv_{k}") for k in range(KD)]
        wo_sb = [per.tile([P, D], bf, name=f"wo_{k}") for k in range(HL * DH // P)]
        mk_sb = per.tile([P, P], bf)
        # ones row for the K=1 denominator-broadcast matmuls
        ones_sb = per.tile([1, 64], bf, name="ones")
        xc0_sb = [per.tile([P, TCH], bf, name=f"xc0_{k}") for k in range(KD)]
        xr_sb = [per.tile([P, 3 * TCH], bf, name=f"xr_{k}") for k in range(KD)]

        # Q^T per head pair: partitions 0:64 = even head dims, 64:128 = odd.
        # All per-index tiles so Tile's dependency tracking stays exact.
        qt_sb = {
            (m, cc): per.tile([P, TCH], bf, name=f"qt_{m}_{cc}")
            for m in range(HP)
            for cc in range(NC4)
        }
        kt_sb = {
            (m, cc): per.tile([P, TCH], bf, name=f"kt_{m}_{cc}")
            for m in range(HP)
            for cc in range(NC4)
        }
        v_sb = [per.tile([P, HL, 66], bf, name=f"v_{tt}") for tt in range(NTT)]
        yt_sb = {
            (m, cc): per.tile([P, TCH], bf, name=f"yt_{m}_{cc}")
            for m in range(HP)
            for cc in range(NC4)
        }

        # ---- loads, ordered so chunk-0 QKV proj can start early ----
        nc.sync.dma_start(out=mk_sb[:], in_=mk[:, :])
        for k in range(KD):
            nc.sync.dma_start(
                out=xc0_sb[k][:], in_=xT[k * P : (k + 1) * P, 0:TCH]
            )
            nc.sync.dma_start(out=wq_sb[k][:], in_=wq[k * P : (k + 1) * P, :])
        for k in range(KD):
            nc.sync.dma_start(out=wk_sb[k][:], in_=wk[k * P : (k + 1) * P, :])
        for k in range(KD):
            nc.sync.dma_start(out=wv_sb[k][:], in_=wv[k * P : (k + 1) * P, :])
        for k in range(KD):
            nc.sync.dma_start(
                out=xr_sb[k][:], in_=xT[k * P : (k + 1) * P, TCH:T]
            )
        for k in range(HL * DH // P):
            nc.sync.dma_start(out=wo_sb[k][:], in_=wo[k * P : (k + 1) * P, :])

        nc.vector.memset(ones_sb[:, :], 1.0)
        for tt in range(NTT):
            nc.vector.memset(v_sb[tt][:, :, 64:65], 1.0)

        def xap(k, cc, c0, c1):
            """x^T tile slice for contraction tile k, columns [c0,c1) of
            chunk cc."""
            if cc == 0:
                return xc0_sb[k][:, c0:c1]
            base = (cc - 1) * TCH
            return xr_sb[k][:, base + c0 : base + c1]

        pt_pool = ctx.enter_context(tc.tile_pool(name="ptp", bufs=8))
        sm_pool = ctx.enter_context(tc.tile_pool(name="smp", bufs=4))
        out_pool = ctx.enter_context(tc.tile_pool(name="outp", bufs=2))

        def gen_proj(cc):
            """QKV projections for chunk cc; yields after each matmul."""
            for m in range(HP):
                msl = slice(m * P, (m + 1) * P)
                pq = psum.tile([P, TCH], f32, tag="pp", bufs=2, name=f"pq{cc}_{m}")
                for k in range(KD):
                    nc.tensor.matmul(
                        out=pq[:],
                        lhsT=wq_sb[k][:, msl],
                        rhs=xap(k, cc, 0, TCH),
                        start=(k == 0),
                        stop=(k == KD - 1),
                    )
                    yield
                nc.vector.tensor_copy(out=qt_sb[(m, cc)][:, :], in_=pq[:])
                pk = psum.tile([P, TCH], f32, tag="pp", bufs=2, name=f"pk{cc}_{m}")
                for k in range(KD):
                    nc.tensor.matmul(
                        out=pk[:],
                        lhsT=wk_sb[k][:, msl],
                        rhs=xap(k, cc, 0, TCH),
                        start=(k == 0),
                        stop=(k == KD - 1),
                    )
                    yield
                nc.vector.tensor_copy(out=kt_sb[(m, cc)][:, :], in_=pk[:])
            for tt in range(4 * cc, 4 * cc + 4):
                tl = tt - 4 * cc
                pv = psum.tile([P, TCH], f32, tag="pp", bufs=2, name=f"pv{tt}")
                for k in range(KD):
                    nc.tensor.matmul(
                        out=pv[:],
                        lhsT=xap(k, cc, tl * P, (tl + 1) * P),
                        rhs=wv_sb[k][:],
                        start=(k == 0),
                        stop=(k == KD - 1),
                    )
                    yield
                nc.vector.tensor_copy(
                    out=v_sb[tt][:, :, 0:64],
                    in_=pv.rearrange("p (h d) -> p h d", h=HL),
                )

        def gen_oproj(chunks, korder=None, dma_eng=None):
            """Output projection for the given chunks; yields per matmul."""
            ks = list(korder or range(HL * DH // P))
            for c2 in chunks:
                for tt in range(4 * c2, 4 * c2 + 4):
                    ob = out_pool.tile([P, D], f32, tag="ob", name=f"ob{tt}")
                    for n2 in range(2):
                        po = psum.tile(
                            [P, TCH], f32, tag="pp", bufs=2, name=f"po{tt}_{n2}"
                        )
                        for i, k in enumerate(ks):
                            tl = tt - 4 * c2
                            nc.tensor.matmul(
                                out=po[:],
                                lhsT=yt_sb[(k, c2)][:, tl * P : (tl + 1) * P],
                                rhs=wo_sb[k][:, n2 * TCH : (n2 + 1) * TCH],
                                start=(i == 0),
                                stop=(i == len(ks) - 1),
                            )
                            yield
                        nc.vector.tensor_copy(
                            out=ob[:, n2 * TCH : (n2 + 1) * TCH], in_=po[:]
                        )
                    (dma_eng or nc.sync).dma_start(
                        out=out[tt * P : (tt + 1) * P, :], in_=ob[:]
                    )

        # projections for chunk 0 run unzipped up front (also warms the PE)
        for _ in gen_proj(0):
            pass

        # Global filler queue: a list of (tag, generator) consumed ~2 matmuls
        # per attention step; before attention chunk c its projections must be
        # fully emitted (Tile orders by program order), so drain through the
        # matching tag at each chunk start. O-proj generators are appended as
        # soon as their chunk's attention completes.
        fillq = [(("proj", cc), gen_proj(cc)) for cc in range(1, NC4)]

        def fill(n):
            done = 0
            while done < n and fillq:
                try:
                    next(fillq[0][1])
                    done += 1
                except StopIteration:
                    fillq.pop(0)
            return done

        def drain_through(tag):
            while fillq and any(t == tag for t, _ in fillq):
                try:
                    next(fillq[0][1])
                except StopIteration:
                    fillq.pop(0)

        FILL_PER_STEP = 2

        def make_norm(hp_n, psys_n, c_n, then_append=None):
            """Deferred normalize for head pair hp_n of chunk c_n:
            y^T = psy[0:64] * (1/psy[64]), the reciprocal computed on the
            (otherwise idle-ish) scalar engine and broadcast across
            partitions by K=1 matmuls col-tiled to (0,0)/(0,64).  Runs one
            head pair late so the PE never waits on the chain."""

            def emit():
                bc = psum.tile([P, TCH], f32, tag="pp", bufs=2, name="bc")
                with nc.allow_low_precision(reason="bf16 recip feeds bcast"):
                    for par in (0, 1):
                        # 1/d as exp(-ln d): two cheap scalar-engine LUT ops
                        # (DVE reciprocal is ~10 cyc/elem and this row sits
                        # on a single partition lane)
                        ld = sm_pool.tile([1, TCH], f32, tag=f"ld{par}", name="ld")
                        nc.scalar.activation(
                            out=ld[:, :],
                            in_=psys_n[par][64:65, :],
                            func=mybir.ActivationFunctionType.Ln,
                        )
                        rr = sm_pool.tile([1, TCH], bf, tag=f"rr{par}", name="rr")
                        nc.scalar.activation(
                            out=rr[:, :],
                            in_=ld[:, :],
                            func=mybir.ActivationFunctionType.Exp,
                            scale=-1.0,
                        )
                        nc.tensor.matmul(
                            out=bc[64 * par : 64 * par + 64, :],
                            lhsT=ones_sb[:, :],
                            rhs=rr[:, :],
                            start=True,
                            stop=True,
                            tile_position=(0, 64 * par),
                        )
                rb = sm_pool.tile([P, TCH], f32, tag="rb", name="rb")
                nc.vector.tensor_copy(out=rb[:, :], in_=bc[:, :])
                for par in (0, 1):
                    rows = slice(64 * par, 64 * par + 64)
                    nc.vector.tensor_mul(
                        yt_sb[(hp_n, c_n)][rows, :],
                        psys_n[par][0:64, :],
                        rb[rows, :],
                    )
                if then_append is not None:
                    fillq.append(then_append)

            return emit

        pending = []

        # ---- attention: per chunk, all head pairs, with filler zipped in ----
        for c in range(NC4):
            n_st = 4 * c + 4
            drain_through(("proj", c))

            # last chunk: run hp=3 first so the final O-proj (k emitted in the
            # same rotated order) is never blocked on the last norm chain
            hporder = [3, 0, 1, 2] if c == NC4 - 1 else list(range(HP))
            for hp in hporder:
                pts = {}
                psys = {}

                def emit_av(st, hp=hp, pts=pts, psys=psys, n_st=n_st):
                    pt, base, lo = pts[st]
                    for par in (0, 1):
                        if st == 0:
                            psys[par] = psum.tile(
                                [65, TCH], f32, tag="py", bufs=2, name=f"psy{par}"
                            )
                        nc.tensor.matmul(
                            out=psys[par][:, lo:TCH],
                            lhsT=v_sb[st][:, 2 * hp + par, 0:65],
                            rhs=pt[:, base + par, lo:TCH],
                            start=(st == 0),
                            stop=(st == n_st - 1),
                        )

                for st in range(n_st):
                    kd = st - 4 * c  # >=0 on causal-diagonal s-tiles
                    lo = max(kd, 0) * P
                    pss = psum.tile([P, 2 * TCH], f32, tag="ps2", bufs=2, name="pss")
                    # both heads of the pair packed into disjoint PE
                    # row-halves -> the two matmuls run concurrently
                    for par in (0, 1):
                        rows = slice(64 * par, 64 * par + 64)
                        nc.tensor.matmul(
                            out=pss[:, par * TCH + lo : (par + 1) * TCH],
                            lhsT=kt_sb[(hp, st // 4)][
                                rows, (st % 4) * P : (st % 4 + 1) * P
                            ],
                            rhs=qt_sb[(hp, c)][rows, lo:TCH],
                            start=True,
                            stop=True,
                            tile_position=(64 * par, 0),
                        )
                    pt = pt_pool.tile([P, 2, TCH], bf, tag="pt", name="pt")
                    nc.scalar.activation(
                        out=pt[:, :, lo:TCH],
                        in_=pss.rearrange("p (a b) -> p a b", a=2)[:, :, lo:TCH],
                        func=Exp,
                        scale=1.0 / np.sqrt(DH),
                    )
                    if kd >= 0:
                        for par in (0, 1):
                            nc.vector.tensor_mul(
                                pt[:, par, lo : lo + P],
                                pt[:, par, lo : lo + P],
                                mk_sb[:],
                            )
                    pts[st] = (pt, 0, lo)
                    if st >= AVLAG:
                        emit_av(st - AVLAG)
                    if st == 1:
                        while pending:
                            pending.pop(0)()
                    fill(FILL_PER_STEP)
                for st in range(n_st - AVLAG, n_st):
                    emit_av(st)
                pending.append(make_norm(hp, psys, c))
            # this chunk's output projection becomes filler once its last
            # head pair's (deferred) normalize has been emitted
            if c < NC4 - 1:
                pending[-1] = make_norm(
                    hporder[-1], psys, c, then_append=(("oproj", c), gen_oproj([c]))
                )

        # flush the last normalize, drain remaining filler, then the last
        # chunk's output projection
        while pending:
            pending.pop(0)()
        while fill(64):
            pass
        for _ in gen_oproj([3], korder=[3, 0, 1, 2], dma_eng=nc.scalar):
            pass

    _split_waits(nc, mybir, 1)
    _CACHE["nc"] = nc
    return nc


def kernel(x, Wq, Wk, Wv, Wo):
    from concourse.bass_utils import run_bass_kernel_spmd

    nc = _build()
    bf16 = ml_dtypes.bfloat16

    band = np.tril(np.ones((P, P), np.float32)).T.astype(bf16)  # band[s,j]=s<=j
    xTs = [np.ascontiguousarray(x[b].T).astype(bf16) for b in range(B)]
    in_maps = []
    for c in range(8):
        b, hg = divmod(c, 2)
        sl = slice(512 * hg, 512 * hg + 512)
        in_maps.append(
            {
                "xT": xTs[b],
                "wq": np.ascontiguousarray(Wq[sl, :].T).astype(bf16),
                "wk": np.ascontiguousarray(Wk[sl, :].T).astype(bf16),
                "wv": np.ascontiguousarray(Wv[sl, :].T).astype(bf16),
                "wo": np.ascontiguousarray(Wo[:, sl].T).astype(bf16),
                "mask": band,
            }
        )
    res = None
    for attempt in range(4):
        try:
            res = run_bass_kernel_spmd(nc, in_maps, list(range(8)))
            break
        except Exception:
            # transient NRT_EXEC_UNIT_UNRECOVERABLE has been observed on the
            # first execution of a freshly loaded NEFF; retry a few times
            if attempt == 3:
                raise
            import time

            time.sleep(3)
    _CACHE["exec_time_ns"] = res.exec_time_ns
    outp = np.empty((B, T, D), np.float32)
    for b in range(B):
        outp[b] = res.results[2 * b]["out"] + res.results[2 * b + 1]["out"]
    return outp


# revision 17
# speedup vs baseline: 1.2948x; 1.0221x over previous
"""Causal self-attention on 8 Trainium2 NeuronCores.

Sharding: 8 cores = 4 batches x 2 head-groups (8 heads each).
Each core runs an identical SPMD program:
  - QKV projections for its head group (weights pre-transposed + bf16 on host)
  - causal attention computed in transposed-score layout S^T[s, t] so the
    AV matmul consumes P^T directly (no on-chip transposes at all)
  - softmax denominators come for free from a ones-column appended to V
  - row-sharded Wo projection produces a partial output; the two cores of a
    batch are summed on the host during unsharding.

Schedule: the two heads of a pair run their QK^T matmuls as K=64 matmuls
packed into disjoint PE row-halves (tile_position (0,0) / (64,0)); the HW
runs same-row-group-disjoint matmuls concurrently, so a packed pair costs
one matmul of streaming time.  AV trails the exp by AVLAG steps and
projection/output-projection "filler" matmuls keep the PE stream dense
while ScalarE computes the softmax exps.  The softmax denominator
reciprocals are broadcast across partitions with a tiny K=2 selector
matmul (no DRAM bounce), so the last chunk's normalize no longer blocks
the final output projection on DMA round trips.

B=4, T=2048, D=1024, H=16, dh=64.
"""

import numpy as np
import ml_dtypes

B, T, D = 4, 2048, 1024
P = 128
KD = D // P  # 8 contraction tiles for the input dim
HL = 8  # heads per core
HP = HL // 2  # head pairs per core (pair shares a 128-partition tile)
DH = 64
TCH = 512  # t-chunk (psum bank width in fp32)
NC4 = T // TCH  # 4 chunks
NTT = T // P  # 16 t-tiles
AVLAG = 2  # AV trails QK^T by this many s-tiles (hides exp latency)

_CACHE = {}


def _split_waits(nc, mybir, limit=1):
    """walrus in this container accepts at most one sem-wait per instruction;
    hoist extra waits onto preceding NoOps on the same engine."""
    cnt = 0
    for bb in nc.main_func.blocks:
        newlist = []
        for inst in bb.instructions:
            si = inst.sync_info
            if si is not None and len(si.on_wait) > limit:
                waits = list(si.on_wait)
                extra, keep = waits[:-limit], waits[-limit:]
                for w in extra:
                    cnt += 1
                    nop = mybir.InstNoOp(name=f"WSPLIT-{cnt}")
                    nop.engine = inst.engine
                    nop.sync_info = mybir.SyncInfo(on_wait=[w], on_update=[])
                    newlist.append(nop)
                inst.sync_info = mybir.SyncInfo(
                    on_wait=keep, on_update=list(si.on_update)
                )
            newlist.append(inst)
        bb.instructions[:] = newlist
    return cnt


def _build():
    if "nc" in _CACHE:
        return _CACHE["nc"]

    from contextlib import ExitStack

    import concourse.bass as bass
    import concourse.tile as tile
    from concourse import mybir

    f32 = mybir.dt.float32
    bf = mybir.dt.bfloat16
    Exp = mybir.ActivationFunctionType.Exp

    nc = bass.Bass()
    xT = nc.declare_dram_parameter("xT", [D, T], bf, isOutput=False)
    wq = nc.declare_dram_parameter("wq", [D, HL * DH], bf, isOutput=False)
    wk = nc.declare_dram_parameter("wk", [D, HL * DH], bf, isOutput=False)
    wv = nc.declare_dram_parameter("wv", [D, HL * DH], bf, isOutput=False)
    wo = nc.declare_dram_parameter("wo", [HL * DH, D], bf, isOutput=False)
    mk = nc.declare_dram_parameter("mask", [P, P], bf, isOutput=False)
    out = nc.declare_dram_parameter("out", [T, D], f32, isOutput=True)

    with tile.TileContext(nc) as tc, ExitStack() as ctx:
        psum = ctx.enter_context(tc.tile_pool(name="psum", bufs=1, space="PSUM"))
        per = ctx.enter_context(tc.tile_pool(name="per", bufs=1))

        # one consolidated DMA per input tensor (each dma_start costs ~600ns
        # of serial dispatch on its queue engine), chunk-0 x split out so the
        # first projection only waits on ~2 MB of the 8.4 MB load
        wq_sb = per.tile([P, KD, HL * DH], bf, name="wq_sb")
        wk_sb = per.tile([P, KD, HL * DH], bf, name="wk_sb")
        wv_sb = per.tile([P, KD, HL * DH], bf, name="wv_sb")
        wo_sb = per.tile([P, HL * DH // P, D], bf, name="wo_sb")
        mk_sb = per.tile([P, P], bf)
        # ones row for the K=1 denominator-broadcast matmuls
        ones_sb = per.tile([1, 64], bf, name="ones")
        xc0_sb = per.tile([P, KD, TCH], bf, name="xc0_sb")
        xr_sb = per.tile([P, KD, 3 * TCH], bf, name="xr_sb")

        # Q^T per head pair: partitions 0:64 = even head dims, 64:128 = odd.
        # All per-index tiles so Tile's dependency tracking stays exact.
        qt_sb = {
            (m, cc): per.tile([P, TCH], bf, name=f"qt_{m}_{cc}")
            for m in range(HP)
            for cc in range(NC4)
        }
        kt_sb = {
            (m, cc): per.tile([P, TCH], bf, name=f"kt_{m}_{cc}")
            for m in range(HP)
            for cc in range(NC4)
        }
        v_sb = [per.tile([P, HL, 66], bf, name=f"v_{tt}") for tt in range(NTT)]
        yt_sb = {
            (m, cc): per.tile([P, TCH], bf, name=f"yt_{m}_{cc}")
            for m in range(HP)
            for cc in range(NC4)
        }

        # ---- loads: 7 consolidated DMAs, dispatch split across the sync
        # and (otherwise idle) gpsimd queues, ordered by first use ----
        NW = HL * DH
        nc.sync.dma_start(out=mk_sb[:], in_=mk[:, :])
        nc.sync.dma_start(
            out=xc0_sb[:, :, :],
            in_=bass.AP(tensor=xT, offset=0, ap=[[T, P], [P * T, KD], [1, TCH]]),
        )
        nc.gpsimd.dma_start(
            out=wq_sb[:, :, :],
            in_=bass.AP(tensor=wq, offset=0, ap=[[NW, P], [P * NW, KD], [1, NW]]),
        )
        nc.sync.dma_start(
            out=wk_sb[:, :, :],
            in_=bass.AP(tensor=wk, offset=0, ap=[[NW, P], [P * NW, KD], [1, NW]]),
        )
        nc.gpsimd.dma_start(
            out=wv_sb[:, :, :],
            in_=bass.AP(tensor=wv, offset=0, ap=[[NW, P], [P * NW, KD], [1, NW]]),
        )
        nc.sync.dma_start(
            out=xr_sb[:, :, :],
            in_=bass.AP(
                tensor=xT, offset=TCH, ap=[[T, P], [P * T, KD], [1, 3 * TCH]]
            ),
        )
        nc.gpsimd.dma_start(
            out=wo_sb[:, :, :],
            in_=bass.AP(tensor=wo, offset=0, ap=[[D, P], [P * D, HL * DH // P], [1, D]]),
        )

        nc.vector.memset(ones_sb[:, :], 1.0)
        for tt in range(NTT):
            nc.vector.memset(v_sb[tt][:, :, 64:65], 1.0)

        def xap(k, cc, c0, c1):
            """x^T tile slice for contraction tile k, columns [c0,c1) of
            chunk cc."""
            if cc == 0:
                return xc0_sb[:, k, c0:c1]
            base = (cc - 1) * TCH
            return xr_sb[:, k, base + c0 : base + c1]

        pt_pool = ctx.enter_context(tc.tile_pool(name="ptp", bufs=8))
        sm_pool = ctx.enter_context(tc.tile_pool(name="smp", bufs=4))
        out_pool = ctx.enter_context(tc.tile_pool(name="outp", bufs=2))

        def gen_proj(cc):
            """QKV projections for chunk cc; yields after each matmul."""
            for m in range(HP):
                msl = slice(m * P, (m + 1) * P)
                pq = psum.tile([P, TCH], f32, tag="pp", bufs=2, name=f"pq{cc}_{m}")
                for k in range(KD):
                    nc.tensor.matmul(
                        out=pq[:],
                        lhsT=wq_sb[:, k, msl],
                        rhs=xap(k, cc, 0, TCH),
                        start=(k == 0),
                        stop=(k == KD - 1),
                    )
                    yield
                nc.vector.tensor_copy(out=qt_sb[(m, cc)][:, :], in_=pq[:])
                pk = psum.tile([P, TCH], f32, tag="pp", bufs=2, name=f"pk{cc}_{m}")
                for k in range(KD):
                    nc.tensor.matmul(
                        out=pk[:],
                        lhsT=wk_sb[:, k, msl],
                        rhs=xap(k, cc, 0, TCH),
                        start=(k == 0),
                        stop=(k == KD - 1),
                    )
                    yield
                nc.vector.tensor_copy(out=kt_sb[(m, cc)][:, :], in_=pk[:])
            for tt in range(4 * cc, 4 * cc + 4):
                tl = tt - 4 * cc
                pv = psum.tile([P, TCH], f32, tag="pp", bufs=2, name=f"pv{tt}")
                for k in range(KD):
                    nc.tensor.matmul(
                        out=pv[:],
                        lhsT=xap(k, cc, tl * P, (tl + 1) * P),
                        rhs=wv_sb[:, k, :],
                        start=(k == 0),
                        stop=(k == KD - 1),
                    )
                    yield
                nc.vector.tensor_copy(
                    out=v_sb[tt][:, :, 0:64],
                    in_=pv.rearrange("p (h d) -> p h d", h=HL),
                )

        def gen_oproj(chunks, korder=None, dma_eng=None, big=False):
            """Output projection for the given chunks; yields per matmul.
            big=True uses the (idle by then) 2-bank ps2 psum ring with both
            n2 halves in one tile and the last korder entry needed only at
            matmul 7 of 8 -- for the final chunk, whose last yt arrives
            late from the deferred normalize."""
            ks = list(korder or range(HL * DH // P))
            for c2 in chunks:
                for tt in range(4 * c2, 4 * c2 + 4):
                    tl = tt - 4 * c2
                    ob = out_pool.tile([P, D], f32, tag="ob", name=f"ob{tt}")
                    if big:
                        po = psum.tile(
                            [P, 2 * TCH], f32, tag="ps2", bufs=2, name=f"po{tt}"
                        )
                        for i, k in enumerate(ks):
                            for n2 in range(2):
                                nc.tensor.matmul(
                                    out=po[:, n2 * TCH : (n2 + 1) * TCH],
                                    lhsT=yt_sb[(k, c2)][:, tl * P : (tl + 1) * P],
                                    rhs=wo_sb[:, k, n2 * TCH : (n2 + 1) * TCH],
                                    start=(i == 0),
                                    stop=(i == len(ks) - 1),
                                )
                                yield
                        nc.vector.tensor_copy(out=ob[:], in_=po[:])
                    else:
                        for n2 in range(2):
                            po = psum.tile(
                                [P, TCH], f32, tag="pp", bufs=2, name=f"po{tt}_{n2}"
                            )
                            for i, k in enumerate(ks):
                                nc.tensor.matmul(
                                    out=po[:],
                                    lhsT=yt_sb[(k, c2)][:, tl * P : (tl + 1) * P],
                                    rhs=wo_sb[:, k, n2 * TCH : (n2 + 1) * TCH],
                                    start=(i == 0),
                                    stop=(i == len(ks) - 1),
                                )
                                yield
                            nc.vector.tensor_copy(
                                out=ob[:, n2 * TCH : (n2 + 1) * TCH], in_=po[:]
                            )
                    (dma_eng or nc.sync).dma_start(
                        out=out[tt * P : (tt + 1) * P, :], in_=ob[:]
                    )

        # projections for chunk 0 run unzipped up front (also warms the PE)
        for _ in gen_proj(0):
            pass

        # Global filler queue: a list of (tag, generator) consumed ~2 matmuls
        # per attention step; before attention chunk c its projections must be
        # fully emitted (Tile orders by program order), so drain through the
        # matching tag at each chunk start. O-proj generators are appended as
        # soon as their chunk's attention completes.
        fillq = [(("proj", cc), gen_proj(cc)) for cc in range(1, NC4)]

        def fill(n):
            done = 0
            while done < n and fillq:
                try:
                    next(fillq[0][1])
                    done += 1
                except StopIteration:
                    fillq.pop(0)
            return done

        def drain_through(tag):
            while fillq and any(t == tag for t, _ in fillq):
                try:
                    next(fillq[0][1])
                except StopIteration:
                    fillq.pop(0)

        FILL_PER_STEP = 2

        def make_norm(hp_n, psys_n, c_n, then_append=None):
            """Deferred normalize for head pair hp_n of chunk c_n:
            y^T = psy[0:64] * (1/psy[64]), the reciprocal computed on the
            (otherwise idle-ish) scalar engine and broadcast across
            partitions by K=1 matmuls col-tiled to (0,0)/(0,64).  Runs one
            head pair late so the PE never waits on the chain."""

            def emit():
                bc = psum.tile([P, TCH], f32, tag="pp", bufs=2, name="bc")
                with nc.allow_low_precision(reason="bf16 recip feeds bcast"):
                    for par in (0, 1):
                        # 1/d as exp(-ln d): two cheap scalar-engine LUT ops
                        # (DVE reciprocal is ~10 cyc/elem and this row sits
                        # on a single partition lane)
                        ld = sm_pool.tile([1, TCH], f32, tag=f"ld{par}", name="ld")
                        nc.scalar.activation(
                            out=ld[:, :],
                            in_=psys_n[par][64:65, :],
                            func=mybir.ActivationFunctionType.Ln,
                        )
                        rr = sm_pool.tile([1, TCH], bf, tag=f"rr{par}", name="rr")
                        nc.scalar.activation(
                            out=rr[:, :],
                            in_=ld[:, :],
                            func=mybir.ActivationFunctionType.Exp,
                            scale=-1.0,
                        )
                        nc.tensor.matmul(
                            out=bc[64 * par : 64 * par + 64, :],
                            lhsT=ones_sb[:, :],
                            rhs=rr[:, :],
                            start=True,
                            stop=True,
                            tile_position=(0, 64 * par),
                        )
                rb = sm_pool.tile([P, TCH], f32, tag="rb", name="rb")
                nc.vector.tensor_copy(out=rb[:, :], in_=bc[:, :])
                for par in (0, 1):
                    rows = slice(64 * par, 64 * par + 64)
                    nc.vector.tensor_mul(
                        yt_sb[(hp_n, c_n)][rows, :],
                        psys_n[par][0:64, :],
                        rb[rows, :],
                    )
                if then_append is not None:
                    fillq.append(then_append)

            return emit

        pending = []

        # ---- attention: per chunk, all head pairs, with filler zipped in ----
        for c in range(NC4):
            n_st = 4 * c + 4
            drain_through(("proj", c))

            # last chunk: run hp=3 first so the final O-proj (k emitted in the
            # same rotated order) is never blocked on the last norm chain
            hporder = [3, 0, 1, 2] if c == NC4 - 1 else list(range(HP))
            for hp in hporder:
                pts = {}
                psys = {}

                def emit_av(st, hp=hp, pts=pts, psys=psys, n_st=n_st):
                    pt, base, lo = pts[st]
                    for par in (0, 1):
                        if st == 0:
                            psys[par] = psum.tile(
                                [65, TCH], f32, tag="py", bufs=2, name=f"psy{par}"
                            )
                        nc.tensor.matmul(
                            out=psys[par][:, lo:TCH],
                            lhsT=v_sb[st][:, 2 * hp + par, 0:65],
                            rhs=pt[:, base + par, lo:TCH],
                            start=(st == 0),
                            stop=(st == n_st - 1),
                        )

                # steps run in pairs: both steps' packed QK matmuls emitted
                # back-to-back so the PE pays the 64-row<->128-row array
                # reconfig drain once per pair instead of once per step
                for j in range(0, n_st, 2):
                    pair = (j, j + 1)
                    pss_l = {}
                    for st in pair:
                        kd = st - 4 * c  # >=0 on causal-diagonal s-tiles
                        lo = max(kd, 0) * P
                        pss = psum.tile(
                            [P, 2 * TCH], f32, tag="ps2", bufs=2, name="pss"
                        )
                        for par in (0, 1):
                            rows = slice(64 * par, 64 * par + 64)
                            nc.tensor.matmul(
                                out=pss[:, par * TCH + lo : (par + 1) * TCH],
                                lhsT=kt_sb[(hp, st // 4)][
                                    rows, (st % 4) * P : (st % 4 + 1) * P
                                ],
                                rhs=qt_sb[(hp, c)][rows, lo:TCH],
                                start=True,
                                stop=True,
                                tile_position=(64 * par, 0),
                            )
                        pss_l[st] = (pss, lo, kd)
                    for st in pair:
                        pss, lo, kd = pss_l[st]
                        pt = pt_pool.tile([P, 2, TCH], bf, tag="pt", name="pt")
                        nc.scalar.activation(
                            out=pt[:, :, lo:TCH],
                            in_=pss.rearrange("p (a b) -> p a b", a=2)[:, :, lo:TCH],
                            func=Exp,
                            scale=1.0 / np.sqrt(DH),
                        )
                        if kd >= 0:
                            for par in (0, 1):
                                nc.vector.tensor_mul(
                                    pt[:, par, lo : lo + P],
                                    pt[:, par, lo : lo + P],
                                    mk_sb[:],
                                )
                        pts[st] = (pt, 0, lo)
                    for st in pair:
                        if st >= AVLAG:
                            emit_av(st - AVLAG)
                        if st == 1:
                            while pending:
                                pending.pop(0)()
                        fill(FILL_PER_STEP)
                for st in range(n_st - AVLAG, n_st):
                    emit_av(st)
                pending.append(make_norm(hp, psys, c))
            # this chunk's output projection becomes filler once its last
            # head pair's (deferred) normalize has been emitted
            if c < NC4 - 1:
                pending[-1] = make_norm(
                    hporder[-1], psys, c, then_append=(("oproj", c), gen_oproj([c]))
                )

        # flush the last normalize, drain remaining filler, then the last
        # chunk's output projection
        while pending:
            pending.pop(0)()
        while fill(64):
            pass
        for _ in gen_oproj([3], korder=[3, 0, 1, 2], dma_eng=nc.scalar, big=True):
            pass

    _split_waits(nc, mybir, 1)
    _CACHE["nc"] = nc
    return nc


def kernel(x, Wq, Wk, Wv, Wo):
    from concourse.bass_utils import run_bass_kernel_spmd

    nc = _build()
    bf16 = ml_dtypes.bfloat16

    band = np.tril(np.ones((P, P), np.float32)).T.astype(bf16)  # band[s,j]=s<=j
    xTs = [np.ascontiguousarray(x[b].T).astype(bf16) for b in range(B)]
    in_maps = []
    for c in range(8):
        b, hg = divmod(c, 2)
        sl = slice(512 * hg, 512 * hg + 512)
        in_maps.append(
            {
                "xT": xTs[b],
                "wq": np.ascontiguousarray(Wq[sl, :].T).astype(bf16),
                "wk": np.ascontiguousarray(Wk[sl, :].T).astype(bf16),
                "wv": np.ascontiguousarray(Wv[sl, :].T).astype(bf16),
                "wo": np.ascontiguousarray(Wo[:, sl].T).astype(bf16),
                "mask": band,
            }
        )
    res = None
    for attempt in range(4):
        try:
            res = run_bass_kernel_spmd(nc, in_maps, list(range(8)))
            break
        except Exception:
            # transient NRT_EXEC_UNIT_UNRECOVERABLE has been observed on the
            # first execution of a freshly loaded NEFF; retry a few times
            if attempt == 3:
                raise
            import time

            time.sleep(3)
    _CACHE["exec_time_ns"] = res.exec_time_ns
    outp = np.empty((B, T, D), np.float32)
    for b in range(B):
        outp[b] = res.results[2 * b]["out"] + res.results[2 * b + 1]["out"]
    return outp
